# revision 1
# baseline (speedup 1.0000x reference)
"""Distributed Bass kernel for the quirky-softmax attention layer on 8 TRN2 NeuronCores.

Reference (N=4096, D=1024, fp32):
    Q = x@Wq + bq; K = x@Wk + bk; V = x@Wv + bv
    S = mask * (Q @ K.T)
    e = exp(S)
    out[i, j] = e[i, j] / rowsum(e)[j]       # quirky: denominator indexed by COLUMN
    return out @ V

Sharding: rows of x across 8 cores (512 rows each). Each core computes
K^T and V for its shard, all-gathers them, computes e^T (scores transposed:
j on partitions, local i on free axis), local row-sums s[i] via a ones-vector
matmul (partition reduction), all-gathers s in 4 chunks (overlapped with
compute), then out^T = V^T_scaled @ e^T. TensorE-facing data is bf16,
accumulation fp32 in PSUM.

DMA queues: scalar (ACT HWDGE) = input params + mask stream;
sync (SP HWDGE) = collective-gated KT/V slab streams + output;
gpsimd (SWDGE) = collective bounce buffers + rowsum combine (accum DMA).
"""

import os
import numpy as np
import ml_dtypes

N = 4096
D = 1024
NC = 8
R = N // NC      # 512 rows per core
P = 128
KT = D // P      # 8 contraction tiles
MT = D // P      # 8 output-feature tiles
JT = N // P      # 32 j tiles
IT = R // P      # 4 i tiles
G = 4            # j-tiles per group (matmul k-subtiles)
NG = JT // G     # 8 groups
NCH = 4          # rowsum all-gather chunks
GPC = NG // NCH  # groups per chunk

LAST_EXEC_NS = None
LAST_RES = None

_cache = {}


def _try_install_ntff_hook():
    """Best-effort registration of the axon NTFF profiling hook (for tracing)."""
    import sys, types

    if "antenv.axon_hooks" in sys.modules:
        return True
    try:
        from trn_agent_boot.trn_boot import _ntff_profile_via_ctypes

        hook = _ntff_profile_via_ctypes("/opt/axon/libaxon_pjrt.so")
        if hook is None:
            return False
        mod = types.ModuleType("antenv.axon_hooks")
        mod.get_axon_ntff_profile_hook = lambda: hook
        mod.set_axon_ntff_profile_hook = lambda h: None
        sys.modules["antenv.axon_hooks"] = mod
        import antenv

        antenv.axon_hooks = mod

        # zero-egress container: the artifact upload would block on network
        from concourse import bass_utils

        bass_utils.upload_artifacts = lambda tmpdir: tmpdir
        return True
    except Exception:
        return False


def _install_neff_cache():
    """Content-keyed NEFF cache: identical BIR -> skip the multi-minute walrus compile."""
    import hashlib
    import shutil

    from concourse import bass2jax, bass_utils

    if getattr(bass_utils, "_neff_cache_installed", False):
        return
    orig = bass_utils.compile_bir_kernel

    def cached(bir_json, tmpdir, neff_name="file.neff"):
        import re

        key = re.sub(rb'"line": \d+', b'"line": 0', bir_json)
        key += os.environ.get("BASS_LDW_OPT", "0").encode()
        h = hashlib.sha256(key).hexdigest()[:24]
        cdir = "/tmp/bass_neff_cache"
        os.makedirs(cdir, exist_ok=True)
        cpath = os.path.join(cdir, h + ".neff")
        if os.path.exists(cpath):
            dst = os.path.join(tmpdir, neff_name)
            shutil.copy(cpath, dst)
            return dst
        p = orig(bir_json, tmpdir, neff_name)
        try:
            shutil.copy(p, cpath)
        except OSError:
            pass
        return p

    bass_utils.compile_bir_kernel = cached
    bass2jax.compile_bir_kernel = cached
    bass_utils._neff_cache_installed = True

    if os.environ.get("BASS_LDW_OPT", "0") == "1":
        orig_run = bass_utils.run_command

        def run_ldw(cmd, *a, **kw):
            cmd = [
                c.replace("--enable-ldw-opt=false", "--enable-ldw-opt=true")
                if isinstance(c, str) else c
                for c in cmd
            ]
            return orig_run(cmd, *a, **kw)

        bass_utils.run_command = run_ldw


def _build():
    import concourse.bacc as bacc
    import concourse.mybir as mybir
    import concourse.tile as tile

    f32 = mybir.dt.float32
    bf16 = mybir.dt.bfloat16
    RG = [list(range(NC))]

    nc = bacc.Bacc("TRN2", target_bir_lowering=False, debug=False, num_devices=NC)

    xT = nc.declare_dram_parameter("xT", [D, R], bf16, isOutput=False)
    maskT = nc.declare_dram_parameter("maskT", [N, R], f32, isOutput=False)
    wq = nc.declare_dram_parameter("wq", [D, D], bf16, isOutput=False)
    wk = nc.declare_dram_parameter("wk", [D, D], bf16, isOutput=False)
    wv = nc.declare_dram_parameter("wv", [D, D], bf16, isOutput=False)
    bq = nc.declare_dram_parameter("bq", [D], f32, isOutput=False)
    bk = nc.declare_dram_parameter("bk", [D], f32, isOutput=False)
    bvb = nc.declare_dram_parameter("bvb", [P, D], f32, isOutput=False)
    outT = nc.declare_dram_parameter("outT", [D, R], f32, isOutput=True)

    with tile.TileContext(nc) as tc:
        with tc.tile_pool(name="dram", bufs=1, space="DRAM") as dram, \
             tc.tile_pool(name="const", bufs=1) as const:
            # K^T gathered in two uneven j-slices: a small first quarter so the
            # first scores tiles start as early as possible, then the rest.
            H1 = P            # j-local quarter width (128)
            H2 = R - P        # remaining width (384)
            kt_in1 = dram.tile([D, H1], bf16)
            kt_ag1 = dram.tile([NC * D, H1], bf16, addr_space="Shared")
            kt_in2 = dram.tile([D, H2], bf16)
            kt_ag2 = dram.tile([NC * D, H2], bf16, addr_space="Shared")
            v_in = dram.tile([R, D], bf16)
            v_ag = dram.tile([N, D], bf16, addr_space="Shared")
            s_in = dram.tile([1, R], f32)
            s_ag = dram.tile([NC, R], f32, addr_space="Shared")

            # wk + xT first, split per k-slice: the first projection matmul only
            # needs the k=0 slices, so PE starts ~15us earlier.
            wk_sb = const.tile([P, KT, D], bf16)
            xt_sb = const.tile([P, KT, R], bf16)
            for k in range(KT):
                nc.scalar.dma_start(wk_sb[:, k, :], wk.ap()[k * P:(k + 1) * P, :])
                nc.scalar.dma_start(xt_sb[:, k, :], xT.ap()[k * P:(k + 1) * P, :])
            bk_sb = const.tile([P, MT], f32)
            nc.scalar.dma_start(bk_sb[:], bk.ap().rearrange("(m p) -> p m", p=P))
            bq_sb = const.tile([P, MT], f32)
            nc.scalar.dma_start(bq_sb[:], bq.ap().rearrange("(m p) -> p m", p=P))
            bv_sb = const.tile([P, D], f32)
            nc.scalar.dma_start(bv_sb[:], bvb.ap())
            ones_sb = const.tile([P, 1], bf16)
            nc.vector.memset(ones_sb[:], 1.0)

            qt_sb = const.tile([P, KT, R], bf16)
            et_sb = const.tile([P, JT, R], bf16)
            r_sb = const.tile([P, JT], f32)

            # ---------------- projections ----------------
            with tc.tile_pool(name="wpool", bufs=2) as wpool, \
                 tc.tile_pool(name="proj_sb", bufs=2) as proj_sb, \
                 tc.tile_pool(name="proj_ps", bufs=5, space="PSUM") as proj_ps:
                # K^T: lhsT = Wk tile, rhs = x^T tile
                kt_sb = proj_sb.tile([P, MT, R], bf16, tag="pout")
                for m in range(MT):
                    ps = proj_ps.tile([P, R], f32, tag="ps", name=f"ps_k{m}")
                    for k in range(KT):
                        nc.tensor.matmul(
                            ps[:], wk_sb[:, k, m * P:(m + 1) * P], xt_sb[:, k, :],
                            start=(k == 0), stop=(k == KT - 1),
                        )
                    nc.vector.tensor_scalar_add(kt_sb[:, m, :], ps[:], bk_sb[:, m:m + 1])
                # K^T all-gather, small quarter first: scores on slice 1 start
                # while slice 2 is still in flight on the collective engine.
                nc.gpsimd.dma_start(
                    kt_in1.rearrange("(m p) j -> p m j", p=P), kt_sb[:, :, 0:H1]
                )
                nc.gpsimd.collective_compute(
                    "AllGather", mybir.AluOpType.bypass, replica_groups=RG,
                    ins=[kt_in1.opt()], outs=[kt_ag1.opt()],
                )
                nc.gpsimd.dma_start(
                    kt_in2.rearrange("(m p) j -> p m j", p=P), kt_sb[:, :, H1:R]
                )
                nc.gpsimd.collective_compute(
                    "AllGather", mybir.AluOpType.bypass, replica_groups=RG,
                    ins=[kt_in2.opt()], outs=[kt_ag2.opt()],
                )

                # V (natural layout): lhsT = x^T tile, rhs = Wv tile
                wv_sb = wpool.tile([P, KT, D], bf16, tag="w")
                nc.scalar.dma_start(wv_sb[:], wv.ap().rearrange("(k p) o -> p k o", p=P))
                v_sb = proj_sb.tile([P, IT, D], bf16, tag="pout")
                for it in range(IT):
                    for c2 in range(2):
                        ps = proj_ps.tile([P, 512], f32, tag="ps", name=f"ps_v{it}_{c2}")
                        for k in range(KT):
                            nc.tensor.matmul(
                                ps[:], xt_sb[:, k, it * P:(it + 1) * P],
                                wv_sb[:, k, c2 * 512:(c2 + 1) * 512],
                                start=(k == 0), stop=(k == KT - 1),
                            )
                        nc.vector.tensor_add(
                            v_sb[:, it, c2 * 512:(c2 + 1) * 512], ps[:],
                            bv_sb[:, c2 * 512:(c2 + 1) * 512],
                        )
                nc.gpsimd.dma_start(v_in.rearrange("(t p) d -> p t d", p=P), v_sb[:])
                nc.gpsimd.collective_compute(
                    "AllGather", mybir.AluOpType.bypass, replica_groups=RG,
                    ins=[v_in.opt()], outs=[v_ag.opt()],
                )

                # Q^T
                wq_sb = wpool.tile([P, KT, D], bf16, tag="w")
                nc.scalar.dma_start(wq_sb[:], wq.ap().rearrange("(k p) o -> p k o", p=P))
                for m in range(MT):
                    ps = proj_ps.tile([P, R], f32, tag="ps", name=f"ps_q{m}")
                    for k in range(KT):
                        nc.tensor.matmul(
                            ps[:], wq_sb[:, k, m * P:(m + 1) * P], xt_sb[:, k, :],
                            start=(k == 0), stop=(k == KT - 1),
                        )
                    nc.vector.tensor_scalar_add(qt_sb[:, m, :], ps[:], bq_sb[:, m:m + 1])

            # ---------------- scores^T + exp + rowsums ----------------
            with tc.tile_pool(name="ktp", bufs=4) as ktp, \
                 tc.tile_pool(name="mp", bufs=8) as mp, \
                 tc.tile_pool(name="tp", bufs=4) as tp, \
                 tc.tile_pool(name="sp2", bufs=2) as sp2, \
                 tc.tile_pool(name="sc_ps", bufs=6, space="PSUM") as sc_ps, \
                 tc.tile_pool(name="s1_ps", bufs=1, space="PSUM") as s1_ps:
                s1 = s1_ps.tile([1, R], f32)

                # processing order: j-local quarter 1 of every rank block
                # (kt_ag1), then the remaining three quarters (kt_ag2).
                order = [c * IT for c in range(NC)] + \
                        [c * IT + tl for c in range(NC) for tl in (1, 2, 3)]

                def rowsum_mm(idx):
                    t = order[idx]
                    nc.tensor.matmul(
                        s1[:], ones_sb[:], et_sb[:, t, :],
                        start=(idx == 0), stop=(idx == JT - 1),
                    )
                    if idx == JT - 1:
                        s_sb = sp2.tile([1, R], f32, tag="s_sb")
                        nc.vector.tensor_copy(s_sb[:], s1[:])
                        nc.gpsimd.dma_start(s_in[:], s_sb[:])
                        nc.gpsimd.collective_compute(
                            "AllGather", mybir.AluOpType.bypass, replica_groups=RG,
                            ins=[s_in.opt()], outs=[s_ag.opt()],
                        )

                idx = 0

                def scores_tile(t, ktc, koff):
                    nonlocal idx
                    ps = sc_ps.tile([P, R], f32, tag="ps", name=f"ps_s{t}")
                    for k in range(KT):
                        nc.tensor.matmul(
                            ps[:], ktc[:, k, koff:koff + P], qt_sb[:, k, :],
                            start=(k == 0), stop=(k == KT - 1),
                        )
                    # lag the rowsum matmul so PE never waits on ACT
                    idx += 1
                    if idx >= 2:
                        rowsum_mm(idx - 2)
                    mt_t = mp.tile([P, R], f32, tag="mask", name=f"mt{t}")
                    nc.scalar.dma_start(mt_t[:], maskT.ap()[t * P:(t + 1) * P, :])
                    msked = tp.tile([P, R], f32, tag="msked", name=f"msk{t}")
                    nc.vector.tensor_mul(msked[:], ps[:], mt_t[:])
                    nc.scalar.activation(
                        et_sb[:, t, :], msked[:], mybir.ActivationFunctionType.Exp
                    )

                for c in range(NC):
                    ktc = ktp.tile([P, KT, H1], bf16, tag="kt1", name=f"ktc1_{c}")
                    nc.sync.dma_start(
                        ktc[:],
                        kt_ag1[c * D:(c + 1) * D, :]
                        .rearrange("(k p) j -> p k j", p=P),
                    )
                    scores_tile(c * IT, ktc, 0)
                for c in range(NC):
                    ktc = ktp.tile([P, KT, H2], bf16, tag="kt2", name=f"ktc2_{c}")
                    ldma = nc.scalar.dma_start if c % 2 else nc.sync.dma_start
                    ldma(
                        ktc[:],
                        kt_ag2[c * D:(c + 1) * D, :]
                        .rearrange("(k p) j -> p k j", p=P),
                    )
                    for tl in (1, 2, 3):
                        scores_tile(c * IT + tl, ktc, (tl - 1) * P)
                rowsum_mm(JT - 1)

                # 1/s in (p, t) layout
                sr = sp2.tile([P, JT], f32, tag="sr")
                nc.gpsimd.dma_start(sr[:], s_ag.rearrange("r (tt p) -> p (r tt)", p=P))
                nc.vector.reciprocal(r_sb[:], sr[:])
                for t in range(JT):
                    nc.vector.tensor_scalar_mul(
                        et_sb[:, t, :], et_sb[:, t, :], r_sb[:, t:t + 1]
                    )

            # ---------------- out^T = V^T @ e^T (j-contraction) ----------------
            with tc.tile_pool(name="vp", bufs=3) as vp, \
                 tc.tile_pool(name="op", bufs=1) as op, \
                 tc.tile_pool(name="out_ps", bufs=1, space="PSUM") as out_ps:
                pso = [out_ps.tile([P, R], f32, name=f"pso{m}") for m in range(MT)]
                for g in range(NG):
                    vt = vp.tile([P, G, D], bf16, tag="v", name=f"vt{g}")
                    nc.sync.dma_start(
                        vt[:],
                        v_ag[g * G * P:(g + 1) * G * P, :]
                        .rearrange("(t p) d -> p t d", p=P),
                    )
                    for tl in range(G):
                        t = g * G + tl
                        for m in range(MT):
                            nc.tensor.matmul(
                                pso[m][:], vt[:, tl, m * P:(m + 1) * P],
                                et_sb[:, t, :],
                                start=(t == 0), stop=(t == JT - 1),
                            )
                ot_sb = op.tile([P, MT, R], f32)
                for m in range(MT):
                    nc.vector.tensor_copy(ot_sb[:, m, :], pso[m][:])
                nc.sync.dma_start(outT.ap().rearrange("(m p) i -> p m i", p=P), ot_sb[:])

    nc.finalize()
    return nc


def _get_nc():
    if "nc" not in _cache:
        _cache["nc"] = _build()
    return _cache["nc"]


def kernel(x, mask, Wq, bq, Wk, bk, Wv, bv):
    global LAST_EXEC_NS
    _install_neff_cache()
    from concourse.bass_utils import run_bass_kernel_spmd

    bf = ml_dtypes.bfloat16
    x = np.asarray(x, dtype=np.float32)
    mask = np.asarray(mask, dtype=np.float32)
    wq_b = np.asarray(Wq, dtype=np.float32).astype(bf)
    wk_b = np.asarray(Wk, dtype=np.float32).astype(bf)
    wv_b = np.asarray(Wv, dtype=np.float32).astype(bf)
    bq_f = np.asarray(bq, dtype=np.float32)
    bk_f = np.asarray(bk, dtype=np.float32)
    bvb = np.ascontiguousarray(
        np.broadcast_to(np.asarray(bv, dtype=np.float32), (P, D))
    )

    in_maps = []
    for c in range(NC):
        rows = slice(c * R, (c + 1) * R)
        in_maps.append({
            "xT": np.ascontiguousarray(x[rows, :].T).astype(bf),
            "maskT": np.ascontiguousarray(mask[rows, :].T),
            "wq": wq_b, "wk": wk_b, "wv": wv_b,
            "bq": bq_f, "bk": bk_f, "bvb": bvb,
        })

    nc = _get_nc()
    trace = os.environ.get("BASS_KERNEL_TRACE", "0") == "1"
    if trace:
        trace = _try_install_ntff_hook()
    res = run_bass_kernel_spmd(
        nc, in_maps, core_ids=list(range(NC)), trace=trace,
        **({"trace_cores": [0]} if trace else {}),
    )
    LAST_EXEC_NS = res.exec_time_ns
    globals()["LAST_RES"] = res
    out = np.concatenate(
        [res.results[c]["outT"].T for c in range(NC)], axis=0
    ).astype(np.float32)
    return out

